# revision 1
# baseline (speedup 1.0000x reference)
"""Cross-attention kernel for Trainium2, 8 NeuronCores.

Sharding: core = (batch b in 0..3) x (head-group hg in 0..1).
Each core computes, for its batch and its 8 heads (512 of the 1024 H cols):
    qT = (Wq_h^T @ query[b]^T)        [512, SQ]   (+bq per-partition)
    kT = (Wk_h^T @ key_value[b]^T)    [512, SKV]  (+bk per-partition)
    v  = key_value[b] @ Wv_h          [SKV, 512]  (stored per kv-tile with a
                                                   ones-column per head: [128, 8*65])
    per head h, per q-chunk:
        scoresT = kT_h^T-slice matmuls -> [kv 128, q]  (PSUM)
        expT    = exp(scoresT / 8)                      (ACT, PSUM->SBUF)
        outT_h  = v_aug_h^T @ expT -> [65, q] PSUM accum over kv tiles;
                  row 64 = softmax denominator (ones column)
        normalize: recip(row64) -> PE broadcast to [65,q] -> DVE multiply
    out_partial = (attn_outT as lhsT) @ Wo_h  -> [SQ, 1024]  natural, DMA out.
Host sums the two head-group partials per batch and adds bv@Wo + bo.

Precision: matmuls run in float32r (fp32 bits, single-pass PE mode, 4x the
rate of strict fp32); the attn-weights x V averaging path runs in fp16 with a
constant exp offset (exp(s/8 - 3), cancels in the normalize) to stay in fp16
range. Softmax skips max-subtraction: |scores| <= ~5 for this problem's scale.
Measured vs the fp32 reference: rel err ~3e-4.
"""

import ml_dtypes
import numpy as np

import concourse.bass as bass
import concourse.mybir as mybir
import concourse.tile as tile
from concourse import bacc
from concourse import bass_utils

FP32 = mybir.dt.float32
FP32R = mybir.dt.float32r  # fp32 bits, single-pass PE mode (4x faster matmul)
F16 = mybir.dt.float16
P = 128

B, SQ, SKV = 4, 2048, 2048
D, H, NH, HD = 1024, 1024, 16, 64
HC = 512          # H columns per core (8 heads)
NHC = 8           # heads per core
VW = HD + 1       # v columns per head incl. ones column


def build_core_program(sq=SQ, skv=SKV, n_devices=8):
    nc = bacc.Bacc(
        "TRN2",
        target_bir_lowering=False,
        debug=False,
        enable_asserts=False,
        num_devices=n_devices,
    )

    xqT = nc.dram_tensor("xqT", (D, sq), FP32R, kind="ExternalInput").ap()
    xkT = nc.dram_tensor("xkT", (D, skv), FP32R, kind="ExternalInput").ap()
    wq = nc.dram_tensor("wq", (D, HC), FP32R, kind="ExternalInput").ap()
    wk = nc.dram_tensor("wk", (D, HC), FP32R, kind="ExternalInput").ap()
    wv = nc.dram_tensor("wv", (D, HC), FP32R, kind="ExternalInput").ap()
    wo = nc.dram_tensor("wo", (HC, D), FP32R, kind="ExternalInput").ap()
    bq = nc.dram_tensor("bq", (HC, 1), FP32, kind="ExternalInput").ap()
    bk = nc.dram_tensor("bk", (HC, 1), FP32, kind="ExternalInput").ap()
    onesd = nc.dram_tensor("onesd", (P, VW), FP32R, kind="ExternalInput").ap()
    onesb = nc.dram_tensor("onesb", (P, NHC), F16, kind="ExternalInput").ap()
    out = nc.dram_tensor("out", (sq, D), FP32, kind="ExternalOutput").ap()

    ND = D // P            # 8 contraction chunks for projections
    NI = HC // P           # 4 Hc tiles
    NQT = sq // P          # q tiles
    NKT = skv // P         # kv tiles
    PC = min(512, sq)      # projection q/kv chunk
    NPCQ = sq // PC
    PCK = min(512, skv)
    NPCK = skv // PCK
    QC = min(1024, sq)     # attention q chunk (2 PSUM banks)
    NQC = sq // QC
    SUB = 512              # matmul moving-operand max for fp32

    EXP = mybir.ActivationFunctionType.Exp

    with nc.allow_low_precision(reason="fp32r matmul pipeline"), tile.TileContext(nc) as tc:
        with tc.tile_pool(name="persist", bufs=1) as persist:
            qT = [persist.tile([P, sq], FP32R, tag=f"qT{i}", name=f"qT{i}") for i in range(NI)]
            kT = [persist.tile([P, skv], FP32R, tag=f"kT{i}", name=f"kT{i}") for i in range(NI)]
            vaug = [persist.tile([P, NHC * VW], F16, tag=f"v{t}", name=f"v{t}") for t in range(NKT)]
            bqs = persist.tile([P, NI], FP32, tag="bqs")
            bks = persist.tile([P, NI], FP32, tag="bks")
            ones65 = persist.tile([1, VW], FP32R, tag="ones65")
            ones8 = persist.tile([P, NHC], F16, tag="ones8")
            nbias = persist.tile([P, 1], FP32, tag="nbias")
            nc.vector.memset(nbias[:], -3.0)

            nc.sync.dma_start(out=ones65[:], in_=onesd[0:1, :])
            nc.sync.dma_start(out=ones8[:], in_=onesb[:])
            for i in range(NI):
                nc.sync.dma_start(out=bqs[:, i : i + 1], in_=bq[i * P : (i + 1) * P, :])
                nc.sync.dma_start(out=bks[:, i : i + 1], in_=bk[i * P : (i + 1) * P, :])

            # ---------------- projections ----------------
            with (
                tc.tile_pool(name="wts", bufs=1) as wts,
                tc.tile_pool(name="xs", bufs=12) as xs,
                tc.tile_pool(name="ppsum", bufs=4, space=bass.MemorySpace.PSUM) as ppsum,
                tc.tile_pool(name="vpsum", bufs=2, space=bass.MemorySpace.PSUM) as vpsum,
            ):
                wq_sb = [wts.tile([P, HC], FP32R, tag=f"wq{d}", name=f"wq{d}") for d in range(ND)]
                wk_sb = [wts.tile([P, HC], FP32R, tag=f"wk{d}", name=f"wk{d}") for d in range(ND)]
                wv_sb = [wts.tile([P, HC], FP32R, tag=f"wv{d}", name=f"wv{d}") for d in range(ND)]
                for d in range(ND):
                    nc.sync.dma_start(out=wq_sb[d][:], in_=wq[d * P : (d + 1) * P, :])
                    nc.sync.dma_start(out=wk_sb[d][:], in_=wk[d * P : (d + 1) * P, :])
                    nc.sync.dma_start(out=wv_sb[d][:], in_=wv[d * P : (d + 1) * P, :])

                # kT + v projections share the xkT chunk stream
                for c in range(NPCK):
                    xk_c = []
                    for d in range(ND):
                        t = xs.tile([P, PCK], FP32R, tag="x", name="xk")
                        nc.sync.dma_start(
                            out=t[:], in_=xkT[d * P : (d + 1) * P, c * PCK : (c + 1) * PCK]
                        )
                        xk_c.append(t)
                    for i in range(NI):
                        ps = ppsum.tile([P, PCK], FP32, tag="pp")
                        for d in range(ND):
                            nc.tensor.matmul(
                                ps[:],
                                wk_sb[d][:, i * P : (i + 1) * P],
                                xk_c[d][:],
                                start=(d == 0),
                                stop=(d == ND - 1),
                            )
                        nc.vector.tensor_scalar_add(
                            out=kT[i][:, c * PCK : (c + 1) * PCK],
                            in0=ps[:],
                            scalar1=bks[:, i : i + 1],
                        )
                    # v: natural orientation [kv-tile, Hc] accum over d
                    for tt in range(PCK // P):
                        kvt = c * (PCK // P) + tt
                        ps = vpsum.tile([P, HC], FP32, tag="pv")
                        for d in range(ND):
                            nc.tensor.matmul(
                                ps[:],
                                xk_c[d][:, tt * P : (tt + 1) * P],
                                wv_sb[d][:],
                                start=(d == 0),
                                stop=(d == ND - 1),
                            )
                        vv = vaug[kvt].rearrange("p (h w) -> p h w", w=VW)
                        nc.vector.tensor_copy(out=vv[:, :, HD : HD + 1], in_=ones8.rearrange("p (h w) -> p h w", w=1))
                        for h in range(NHC):
                            nc.vector.tensor_copy(
                                out=vaug[kvt][:, h * VW : h * VW + HD],
                                in_=ps[:, h * HD : (h + 1) * HD],
                            )

                # qT projection: out[Hc-tile, q-chunk] accum over d
                for c in range(NPCQ):
                    xq_c = []
                    for d in range(ND):
                        t = xs.tile([P, PC], FP32R, tag="x", name="xq")
                        nc.sync.dma_start(
                            out=t[:], in_=xqT[d * P : (d + 1) * P, c * PC : (c + 1) * PC]
                        )
                        xq_c.append(t)
                    for i in range(NI):
                        ps = ppsum.tile([P, PC], FP32, tag="pp")
                        for d in range(ND):
                            nc.tensor.matmul(
                                ps[:],
                                wq_sb[d][:, i * P : (i + 1) * P],
                                xq_c[d][:],
                                start=(d == 0),
                                stop=(d == ND - 1),
                            )
                        nc.vector.tensor_scalar_add(
                            out=qT[i][:, c * PC : (c + 1) * PC],
                            in0=ps[:],
                            scalar1=bqs[:, i : i + 1],
                        )

            # ---------------- attention + output projection ----------------
            with (
                tc.tile_pool(name="wop", bufs=1) as wop,
                tc.tile_pool(name="otp", bufs=1) as otp,
                tc.tile_pool(name="esb", bufs=9) as esb,
                tc.tile_pool(name="smalls", bufs=4) as smalls,
            ):
                wo_sb = [wop.tile([P, D], FP32R, tag=f"wo{j}", name=f"wo{j}") for j in range(NI)]
                for j in range(NI):
                    nc.sync.dma_start(out=wo_sb[j][:], in_=wo[j * P : (j + 1) * P, :])
                with (
                    tc.tile_pool(name="scps", bufs=3, space=bass.MemorySpace.PSUM) as scps,
                    tc.tile_pool(name="ovps", bufs=1, space=bass.MemorySpace.PSUM) as ovps,
                    tc.tile_pool(name="ost", bufs=4) as ost,
                ):
                    # q-chunk outer, heads inner: the per-chunk out-proj
                    # interleaves with the next chunk's attention so the PE
                    # never idles long enough for HAM to clock-throttle it.
                    for c in range(NQC):
                        outT = [
                            otp.tile([P, QC], FP32R, tag=f"oT{i}", name=f"oT{i}", bufs=2)
                            for i in range(NI)
                        ]
                        for h in range(NHC):
                            i, r = h // 2, (h % 2) * HD
                            ovt = ovps.tile([VW, QC], FP32, tag="ov")
                            # software-pipeline attnV LAG tiles behind the
                            # scores/exp stream: the PE's static order then
                            # never blocks on ACT latency (exp has a 4-tile
                            # cushion), keeping the PE busy and HAM-warm.
                            LAG = 0
                            ets = {}
                            for tt in range(NKT + LAG):
                                if tt < NKT:
                                    t = tt
                                    sc = scps.tile([P, QC], FP32, tag="sc")
                                    for s in range(0, QC, SUB):
                                        w = min(SUB, QC - s)
                                        nc.tensor.matmul(
                                            sc[:, s : s + w],
                                            kT[i][r : r + HD, t * P : (t + 1) * P],
                                            qT[i][r : r + HD, c * QC + s : c * QC + s + w],
                                            start=True,
                                            stop=True,
                                        )
                                    et = esb.tile([P, QC], F16, tag="e")
                                    nc.scalar.activation(et[:], sc[:], EXP, scale=0.125, bias=nbias[:, 0:1])
                                    ets[t] = et
                                if tt >= LAG:
                                    t = tt - LAG
                                    et = ets.pop(t)
                                    for s in range(0, QC, SUB):
                                        w = min(SUB, QC - s)
                                        nc.tensor.matmul(
                                            ovt[:, s : s + w],
                                            vaug[t][:, h * VW : (h + 1) * VW],
                                            et[:, s : s + w],
                                            start=(t == 0),
                                            stop=(t == NKT - 1),
                                        )
                            # normalize: row 64 of ovt is the denominator.
                            # PE-broadcast the denom row, then approx-recip
                            # on 64 lanes (5x faster; denom well-conditioned)
                            rec = smalls.tile([1, QC], FP32R, tag="rec")
                            nc.vector.tensor_copy(out=rec[:], in_=ovt[HD : HD + 1, :])
                            bc = scps.tile([VW, QC], FP32, tag="sc", name="bc")
                            for s in range(0, QC, SUB):
                                w = min(SUB, QC - s)
                                nc.tensor.matmul(
                                    bc[:, s : s + w],
                                    ones65[:],
                                    rec[:, s : s + w],
                                    start=True,
                                    stop=True,
                                )
                            bcs = esb.tile([HD, QC], FP32, tag="bcs", name="bcs")
                            nc.vector.reciprocal_approx_fast(out=bcs[:], in_=bc[0:HD, :])
                            nc.vector.tensor_mul(
                                out=outT[i][r : r + HD, :],
                                in0=ovt[0:HD, :],
                                in1=bcs[:],
                            )

                        # out-proj for this chunk (psum slots shared with
                        # the scores pool; overlaps next chunk's attention)
                        for m in range(QC // P):
                            qm = c * (QC // P) + m
                            for n in range(D // 512):
                                ps = scps.tile([P, 512], FP32, tag="sc", name="op")
                                for j in range(NI):
                                    nc.tensor.matmul(
                                        ps[:],
                                        outT[j][:, m * P : (m + 1) * P],
                                        wo_sb[j][:, n * 512 : (n + 1) * 512],
                                        start=(j == 0),
                                        stop=(j == NI - 1),
                                    )
                                ot = ost.tile([P, 512], FP32, tag="ot")
                                nc.vector.tensor_copy(out=ot[:], in_=ps[:])
                                nc.sync.dma_start(
                                    out=out[qm * P : (qm + 1) * P, n * 512 : (n + 1) * 512],
                                    in_=ot[:],
                                )

    nc.compile()
    return nc


_CACHED_NC = None


def _get_nc():
    global _CACHED_NC
    if _CACHED_NC is None:
        _CACHED_NC = build_core_program()
    return _CACHED_NC


def make_in_maps(query, key_value, Wq, bq, Wk, bk, Wv, bv, Wo, bo):
    query = np.asarray(query, np.float32)
    key_value = np.asarray(key_value, np.float32)
    Wq = np.asarray(Wq, np.float32)
    Wk = np.asarray(Wk, np.float32)
    Wv = np.asarray(Wv, np.float32)
    Wo = np.asarray(Wo, np.float32)
    bq = np.asarray(bq, np.float32)
    bk = np.asarray(bk, np.float32)

    in_maps = []
    for core in range(8):
        b, hg = core // 2, core % 2
        hs = hg * HC
        in_maps.append(
            {
                "xqT": np.ascontiguousarray(query[b].T),
                "xkT": np.ascontiguousarray(key_value[b].T),
                "wq": np.ascontiguousarray(Wq[:, hs : hs + HC]),
                "wk": np.ascontiguousarray(Wk[:, hs : hs + HC]),
                "wv": np.ascontiguousarray(Wv[:, hs : hs + HC]),
                "wo": np.ascontiguousarray(Wo[hs : hs + HC, :]),
                "bq": np.ascontiguousarray(bq[hs : hs + HC, None]),
                "bk": np.ascontiguousarray(bk[hs : hs + HC, None]),
                "onesd": np.ones((P, VW), np.float32),
                "onesb": np.ones((P, NHC), np.float16),
            }
        )
    return in_maps


def _install_profiling():
    """Reconstruct the NTFF profile hook this container's boot skipped.

    bass_utils' axon trace path wants antenv.axon_hooks (absent here);
    inject a stub module and register the ctypes-based hook from
    trn_agent_boot. Also keep artifacts local (no bucket in container).
    """
    import sys
    import types

    if "antenv.axon_hooks" in sys.modules:
        return
    import antenv  # noqa: F401

    mod = types.ModuleType("antenv.axon_hooks")
    mod._hook = None

    def set_axon_ntff_profile_hook(h):
        mod._hook = h

    def get_axon_ntff_profile_hook():
        return mod._hook

    mod.set_axon_ntff_profile_hook = set_axon_ntff_profile_hook
    mod.get_axon_ntff_profile_hook = get_axon_ntff_profile_hook
    sys.modules["antenv.axon_hooks"] = mod

    from trn_agent_boot.trn_boot import _ntff_profile_via_ctypes

    hook = _ntff_profile_via_ctypes("/opt/axon/libaxon_pjrt.so")
    if hook is not None:
        set_axon_ntff_profile_hook(hook)

    bass_utils.upload_artifacts = lambda tmpdir: tmpdir


def run_device(inputs, trace=False, **kw):
    if trace:
        _install_profiling()
    nc = _get_nc()
    in_maps = make_in_maps(**inputs)
    res = bass_utils.run_bass_kernel_spmd(
        nc, in_maps, list(range(8)), trace=trace, **kw
    )
    return res


def assemble_output(results, Wv_bias_term):
    out = np.zeros((B, SQ, D), np.float32)
    for core in range(8):
        b = core // 2
        out[b] += results[core]["out"]
    out += Wv_bias_term
    return out


def kernel(**inputs):
    res = run_device(inputs)
    bv = np.asarray(inputs["bv"], np.float32)
    bo = np.asarray(inputs["bo"], np.float32)
    Wo = np.asarray(inputs["Wo"], np.float32)
    # attn rows sum to 1, so the bv shift passes straight through attn@v;
    # bv@Wo + bo is added once on the host.
    bias_term = bv @ Wo + bo
    return assemble_output(res.results, bias_term)



# revision 10
# speedup vs baseline: 1.4602x; 1.4602x over previous
"""Cross-attention kernel for Trainium2, 8 NeuronCores.

Sharding: core = (batch b in 0..3) x (head-group hg in 0..1).
Each core computes, for its batch and its 8 heads (512 of the 1024 H cols):
    kT = (Wk_h^T @ key_value[b]^T)    [512, SKV]  (+bk)      bf16
    v  = key_value[b] @ Wv_h          per kv-tile [128, 8*65] fp16
                                      (ones column per head = softmax denom)
    qT = (Wq_h^T @ query[b]^T)        [512, SQ]   (+bq)      bf16
    per head h, per q-chunk (1024):
        scoresT(t) = kT_h(tile t)^T-slice matmuls -> [kv 128, q] PSUM
        expT(t)    = exp(scoresT/8)  (ACT, PSUM->SBUF fp16)
        outT_h     = vaug_h^T @ expT -> [65, q] PSUM accum over kv tiles;
                     row 64 = denominator
        normalize: DVE copy -> SBUF, PE-broadcast denom row, approx-recip,
                   DVE multiply -> outT (bf16)
    out_partial = outT^T-slices @ Wo_h -> [SQ, 1024] fp32, DMA out.
Host sums the two head-group partials per batch and adds bv@Wo + bo.

Performance design (v2): the baseline issued attnV right after its exp with
no pipeline slack, so the PE stalled on the ACT engine every kv-tile and the
HAM clock governor kept the PE at 1.2 GHz (measured 668us = 802816 matmul
rows x 0.833ns).  This version keeps the PE queue saturated:
  - everything except the first few projection units is interleaved as
    "filler" PE work inside the attention slot stream (remaining K/V/Q
    projections early, next-chunk Q projection and previous-chunk out-proj
    late), so the PE never idles long enough for HAM to throttle;
  - attnV lags its exp by LAG slots (deque backlog), exp output buffered in
    a deep fp16 SBUF pool;
  - all matmul inputs are bf16/fp16 (same 1 cycle/row as fp32r but half the
    SBUF/DMA), accumulation stays fp32 in PSUM.
PE total is ~800k matmul rows -> ~335us at 2.4GHz; ACT exp total ~275us
overlaps underneath.
"""

from collections import deque

import ml_dtypes
import numpy as np

import concourse.bass as bass
import concourse.mybir as mybir
import concourse.tile as tile
from concourse import bacc
from concourse import bass_utils

FP32 = mybir.dt.float32
FP32R = mybir.dt.float32r
BF16 = mybir.dt.bfloat16
F16 = mybir.dt.float16
P = 128

B, SQ, SKV = 4, 2048, 2048
D, H, NH, HD = 1024, 1024, 16, 64
HC = 512          # H columns per core (8 heads)
NHC = 8           # heads per core
VW = HD + 1       # v columns per head incl. ones column

ND = D // P       # 8 contraction chunks for projections
NI = HC // P      # 4 Hc tiles
NKT = SKV // P    # 16 kv tiles
QC = 1024         # attention q chunk (2 PSUM banks)
NQC = SQ // QC    # 2
SUB = 512         # matmul moving-operand max
NCK = 4           # kv/q chunks of 512 for x streaming
CK = 512
LAG = 3           # attnV slots behind exp


def build_core_program(n_devices=8):
    nc = bacc.Bacc(
        "TRN2",
        target_bir_lowering=False,
        debug=False,
        enable_asserts=False,
        num_devices=n_devices,
    )

    xqT = nc.dram_tensor("xqT", (D, SQ), BF16, kind="ExternalInput").ap()
    xkT = nc.dram_tensor("xkT", (D, SKV), BF16, kind="ExternalInput").ap()
    wq = nc.dram_tensor("wq", (D, HC), BF16, kind="ExternalInput").ap()
    wk = nc.dram_tensor("wk", (D, HC), BF16, kind="ExternalInput").ap()
    wv = nc.dram_tensor("wv", (D, HC), BF16, kind="ExternalInput").ap()
    wo = nc.dram_tensor("wo", (HC, D), BF16, kind="ExternalInput").ap()
    bq = nc.dram_tensor("bq", (HC, 1), FP32, kind="ExternalInput").ap()
    bk = nc.dram_tensor("bk", (HC, 1), FP32, kind="ExternalInput").ap()
    onesd = nc.dram_tensor("onesd", (VW, VW), FP32R, kind="ExternalInput").ap()
    out = nc.dram_tensor("out", (SQ, D), FP32, kind="ExternalOutput").ap()

    EXP = mybir.ActivationFunctionType.Exp

    with nc.allow_low_precision(reason="bf16/fp16 attention pipeline"), tile.TileContext(nc) as tc:
        with (
            tc.tile_pool(name="persist", bufs=1) as persist,
            tc.tile_pool(name="xq", bufs=16) as xqp,
            tc.tile_pool(name="et", bufs=12) as etp,
            tc.tile_pool(name="ovsb", bufs=3) as ovsbp,
            tc.tile_pool(name="bcsp", bufs=2) as bcsp,
            tc.tile_pool(name="otp", bufs=1) as otp,
            tc.tile_pool(name="ost", bufs=4) as ostp,
            tc.tile_pool(name="scps", bufs=2, space=bass.MemorySpace.PSUM) as scps,
            tc.tile_pool(name="ovps", bufs=1, space=bass.MemorySpace.PSUM) as ovps,
            tc.tile_pool(name="fps", bufs=2, space=bass.MemorySpace.PSUM) as fps,
        ):
            # ---------------- persistent tiles ----------------
            kT = [persist.tile([P, SKV], BF16, tag=f"kT{i}", name=f"kT{i}") for i in range(NI)]
            qTc = [
                [persist.tile([P, QC], BF16, tag=f"qT{c}_{i}", name=f"qT{c}_{i}") for i in range(NI)]
                for c in range(NQC)
            ]
            vaug = [persist.tile([P, NHC * VW], F16, tag=f"v{t}", name=f"v{t}") for t in range(NKT)]
            xk = [
                [persist.tile([P, CK], BF16, tag=f"xk{ck}_{d}", name=f"xk{ck}_{d}") for d in range(ND)]
                for ck in range(NCK)
            ]
            wq_sb = [persist.tile([P, HC], BF16, tag=f"wq{d}", name=f"wq{d}") for d in range(ND)]
            wk_sb = [persist.tile([P, HC], BF16, tag=f"wk{d}", name=f"wk{d}") for d in range(ND)]
            wv_sb = [persist.tile([P, HC], BF16, tag=f"wv{d}", name=f"wv{d}") for d in range(ND)]
            wo_sb = [persist.tile([P, D], BF16, tag=f"wo{j}", name=f"wo{j}") for j in range(NI)]
            bqs = persist.tile([P, NI], FP32, tag="bqs")
            bks = persist.tile([P, NI], FP32, tag="bks")
            ones65 = persist.tile([VW, VW], FP32R, tag="ones65")
            warm = persist.tile([1, 8], FP32, tag="warm")

            # preload the exp activation table during the DMA lead-in
            nc.vector.memset(warm[:], 0.0)
            nc.scalar.activation(warm[:], warm[:], EXP)

            # vaug ones columns (persist; v-proj writes leave them alone)
            for t in range(NKT):
                vv = vaug[t].rearrange("p (h w) -> p h w", w=VW)
                nc.gpsimd.memset(vv[:, :, HD : HD + 1], 1.0)

            # ---------------- DMA issue (priority order) ----------------
            nc.sync.dma_start(out=ones65[:], in_=onesd[:])
            for i in range(NI):
                nc.sync.dma_start(out=bqs[:, i : i + 1], in_=bq[i * P : (i + 1) * P, :])
                nc.sync.dma_start(out=bks[:, i : i + 1], in_=bk[i * P : (i + 1) * P, :])
            for d in range(ND):
                nc.sync.dma_start(out=wk_sb[d][:], in_=wk[d * P : (d + 1) * P, :])
                nc.sync.dma_start(out=xk[0][d][:], in_=xkT[d * P : (d + 1) * P, 0:CK])
            xq_c0 = []
            for d in range(ND):
                nc.sync.dma_start(out=wq_sb[d][:], in_=wq[d * P : (d + 1) * P, :])
                t = xqp.tile([P, CK], BF16, tag="xq", name="xq")
                nc.sync.dma_start(out=t[:], in_=xqT[d * P : (d + 1) * P, 0:CK])
                xq_c0.append(t)
            for d in range(ND):
                nc.sync.dma_start(out=wv_sb[d][:], in_=wv[d * P : (d + 1) * P, :])
            xq_c1 = []
            for d in range(ND):
                t = xqp.tile([P, CK], BF16, tag="xq", name="xq")
                nc.sync.dma_start(out=t[:], in_=xqT[d * P : (d + 1) * P, CK : 2 * CK])
                xq_c1.append(t)
            for ck in range(1, NCK):
                for d in range(ND):
                    nc.sync.dma_start(
                        out=xk[ck][d][:], in_=xkT[d * P : (d + 1) * P, ck * CK : (ck + 1) * CK]
                    )
            for j in range(NI):
                nc.sync.dma_start(out=wo_sb[j][:], in_=wo[j * P : (j + 1) * P, :])

            # xq chunks 2..3 loaded lazily (filler DMA below)
            xq_chunks = {0: xq_c0, 1: xq_c1}

            # ---------------- unit thunks (filler PE work) ----------------
            def kT_unit(i, ck):
                ps = fps.tile([P, SUB], FP32, tag="fp")
                for d in range(ND):
                    nc.tensor.matmul(
                        ps[:],
                        wk_sb[d][:, i * P : (i + 1) * P],
                        xk[ck][d][:],
                        start=(d == 0),
                        stop=(d == ND - 1),
                    )
                nc.vector.tensor_scalar_add(
                    out=kT[i][:, ck * CK : (ck + 1) * CK],
                    in0=ps[:],
                    scalar1=bks[:, i : i + 1],
                )

            def v_unit(t):
                ck, tt = t // 4, t % 4
                ps = fps.tile([P, SUB], FP32, tag="fp")
                for d in range(ND):
                    nc.tensor.matmul(
                        ps[:],
                        xk[ck][d][:, tt * P : (tt + 1) * P],
                        wv_sb[d][:],
                        start=(d == 0),
                        stop=(d == ND - 1),
                    )
                vv = vaug[t].rearrange("p (h w) -> p h w", w=VW)
                nc.vector.tensor_copy(
                    out=vv[:, :, 0:HD],
                    in_=ps.rearrange("p (h w) -> p h w", w=HD),
                )

            def load_xq_chunk(cq):
                tiles = []
                for d in range(ND):
                    t = xqp.tile([P, CK], BF16, tag="xq", name="xq")
                    nc.sync.dma_start(
                        out=t[:], in_=xqT[d * P : (d + 1) * P, cq * CK : (cq + 1) * CK]
                    )
                    tiles.append(t)
                xq_chunks[cq] = tiles

            def qp_unit(c, i, sub):
                cq = c * (QC // CK) + sub  # global 512-chunk index into SQ
                ps = fps.tile([P, SUB], FP32, tag="fp")
                for d in range(ND):
                    nc.tensor.matmul(
                        ps[:],
                        wq_sb[d][:, i * P : (i + 1) * P],
                        xq_chunks[cq][d][:],
                        start=(d == 0),
                        stop=(d == ND - 1),
                    )
                nc.vector.tensor_scalar_add(
                    out=qTc[c][i][:, sub * CK : (sub + 1) * CK],
                    in0=ps[:],
                    scalar1=bqs[:, i : i + 1],
                )

            def op_unit(c, m, n):
                # out-proj: out[q-tile m of chunk c, 512-col n] = sum_j outT^T @ wo
                ps = fps.tile([P, SUB], FP32, tag="fp", name="op")
                for j in range(NI):
                    nc.tensor.matmul(
                        ps[:],
                        outT_of[c][j][:, m * P : (m + 1) * P],
                        wo_sb[j][:, n * SUB : (n + 1) * SUB],
                        start=(j == 0),
                        stop=(j == NI - 1),
                    )
                ot = ostp.tile([P, SUB], FP32, tag="ot")
                nc.vector.tensor_copy(out=ot[:], in_=ps[:])
                qm = c * (QC // P) + m
                nc.sync.dma_start(
                    out=out[qm * P : (qm + 1) * P, n * SUB : (n + 1) * SUB], in_=ot[:]
                )

            # ---------------- pre phase ----------------
            kT_unit(0, 0)
            qp_unit(0, 0, 0)
            qp_unit(0, 0, 1)
            for t in range(4):
                v_unit(t)
            v_emit_slot = {0: -10, 1: -10, 2: -10, 3: -10}

            # ---------------- filler list (c=0 era) ----------------
            fillers = deque()
            fillers.extend(("kT", 0, ck) for ck in range(1, NCK))
            for ck in range(1, NCK):
                fillers.extend(("v", t) for t in range(4 * ck, 4 * ck + 4))
            fillers.extend([("qp", 0, 1, 0), ("qp", 0, 1, 1)])
            fillers.extend(("kT", 1, ck) for ck in range(NCK))
            fillers.extend([("qp", 0, 2, 0), ("qp", 0, 2, 1)])
            fillers.extend(("kT", 2, ck) for ck in range(NCK))
            fillers.extend([("qp", 0, 3, 0), ("qp", 0, 3, 1)])
            fillers.extend(("kT", 3, ck) for ck in range(NCK))
            fillers.extend([("xq", 2), ("qp", 1, 0, 0), ("xq", 3), ("qp", 1, 0, 1)])
            fillers.extend([("qp", 1, 1, 0), ("qp", 1, 1, 1)])
            fillers.extend([("qp", 1, 2, 0), ("qp", 1, 2, 1)])
            fillers.extend([("qp", 1, 3, 0), ("qp", 1, 3, 1)])

            def run_filler(f, slot):
                kind = f[0]
                if kind == "kT":
                    kT_unit(f[1], f[2])
                elif kind == "v":
                    v_unit(f[1])
                    v_emit_slot[f[1]] = slot
                elif kind == "qp":
                    qp_unit(f[1], f[2], f[3])
                elif kind == "xq":
                    load_xq_chunk(f[1])
                elif kind == "op":
                    op_unit(f[1], f[2], f[3])

            # ---------------- attention slot stream ----------------
            outT_of = {}
            et_of = {}
            ovt_of = {}
            sc_slot = {}
            drain_slot = {}
            post = []  # (due_slot, kind, c, h)
            av_q = deque()
            slot = 0

            def emit_sc_exp(c, h, t, slot):
                i, r = h // 2, (h % 2) * HD
                sc = scps.tile([P, QC], FP32, tag="sc")
                for s in range(0, QC, SUB):
                    nc.tensor.matmul(
                        sc[:, s : s + SUB],
                        kT[i][r : r + HD, t * P : (t + 1) * P],
                        qTc[c][i][r : r + HD, s : s + SUB],
                        start=True,
                        stop=True,
                    )
                et = etp.tile([P, QC], F16, tag="et")
                nc.scalar.activation(et[:], sc[:], EXP, scale=0.125)
                et_of[(c, h, t)] = et
                sc_slot[(c, h, t)] = slot

            def emit_av(c, h, t):
                if t == 0:
                    ovt_of[(c, h)] = ovps.tile([VW, QC], FP32, tag="ov", name="ovt")
                ovt = ovt_of[(c, h)]
                et = et_of.pop((c, h, t))
                for s in range(0, QC, SUB):
                    nc.tensor.matmul(
                        ovt[:, s : s + SUB],
                        vaug[t][:, h * VW : (h + 1) * VW],
                        et[:, s : s + SUB],
                        start=(t == 0),
                        stop=(t == NKT - 1),
                    )

            def emit_drain(c, h):
                ovt = ovt_of.pop((c, h))
                ovsb = ovsbp.tile([VW, QC], FP32R, tag="ovsb")
                nc.vector.tensor_copy(out=ovsb[:], in_=ovt[:])
                ovsb_of[(c, h)] = ovsb

            def emit_norm(c, h):
                i, r = h // 2, (h % 2) * HD
                ovsb = ovsb_of.pop((c, h))
                bc = scps.tile([VW, QC], FP32, tag="sc", name="bc")
                for s in range(0, QC, SUB):
                    nc.tensor.matmul(
                        bc[:, s : s + SUB],
                        ones65[HD : HD + 1, :],
                        ovsb[HD : HD + 1, s : s + SUB],
                        start=True,
                        stop=True,
                    )
                bcs = bcsp.tile([HD, QC], FP32, tag="bcs", name="bcs")
                nc.vector.reciprocal_approx_fast(out=bcs[:], in_=bc[0:HD, :])
                nc.vector.tensor_mul(
                    out=outT_of[c][i][r : r + HD, :],
                    in0=ovsb[0:HD, :],
                    in1=bcs[:],
                )
                if h == NHC - 1:
                    # previous chunk fully normalized -> out-proj becomes filler
                    fillers.extend(
                        ("op", c, m, n) for m in range(QC // P) for n in range(D // SUB)
                    )

            ovsb_of = {}

            def av_poppable(slot):
                if not av_q:
                    return False
                c, h, t = av_q[0]
                if slot - sc_slot[(c, h, t)] < LAG:
                    return False
                if c == 0 and slot - v_emit_slot.get(t, 10**9) < 2:
                    return False
                if t == 0:
                    prev = (c, h - 1) if h > 0 else (c - 1, NHC - 1)
                    if prev[1] >= 0 and prev[0] >= 0:
                        ds = drain_slot.get(prev)
                        if ds is None or slot - ds < 2:
                            return False
                return True

            def pop_avs(slot, maxn=2):
                n = 0
                while n < maxn and av_poppable(slot):
                    c, h, t = av_q.popleft()
                    emit_av(c, h, t)
                    n += 1
                    if t == NKT - 1:
                        emit_drain(c, h)
                        drain_slot[(c, h)] = slot
                        post.append((slot + 2, c, h))

            def run_due_posts(slot):
                while post and post[0][0] <= slot:
                    _, c, h = post.pop(0)
                    emit_norm(c, h)

            for c in range(NQC):
                outT_of[c] = [
                    otp.tile([P, QC], BF16, tag=f"oT{j}", name=f"oT{j}", bufs=2)
                    for j in range(NI)
                ]
                for h in range(NHC):
                    for t in range(NKT):
                        run_due_posts(slot)
                        emit_sc_exp(c, h, t, slot)
                        pop_avs(slot)
                        if fillers and (slot < 16 or slot % 2 == 0):
                            run_filler(fillers.popleft(), slot)
                        av_q.append((c, h, t))
                        slot += 1

            # ---------------- tail ----------------
            guard = 0
            while av_q or post:
                run_due_posts(slot)
                pop_avs(slot)
                if fillers:
                    run_filler(fillers.popleft(), slot)
                slot += 1
                guard += 1
                assert guard < 500, "tail drain stuck"
            while fillers:
                run_filler(fillers.popleft(), slot)
                slot += 1

    nc.compile()
    return nc


_CACHED_NC = None


def _get_nc():
    global _CACHED_NC
    if _CACHED_NC is None:
        _CACHED_NC = build_core_program()
    return _CACHED_NC


def make_in_maps(query, key_value, Wq, bq, Wk, bk, Wv, bv, Wo, bo):
    query = np.asarray(query, np.float32)
    key_value = np.asarray(key_value, np.float32)
    Wq = np.asarray(Wq, np.float32)
    Wk = np.asarray(Wk, np.float32)
    Wv = np.asarray(Wv, np.float32)
    Wo = np.asarray(Wo, np.float32)
    bq = np.asarray(bq, np.float32)
    bk = np.asarray(bk, np.float32)
    bf = ml_dtypes.bfloat16

    in_maps = []
    for core in range(8):
        b, hg = core // 2, core % 2
        hs = hg * HC
        in_maps.append(
            {
                "xqT": np.ascontiguousarray(query[b].T).astype(bf),
                "xkT": np.ascontiguousarray(key_value[b].T).astype(bf),
                "wq": np.ascontiguousarray(Wq[:, hs : hs + HC]).astype(bf),
                "wk": np.ascontiguousarray(Wk[:, hs : hs + HC]).astype(bf),
                "wv": np.ascontiguousarray(Wv[:, hs : hs + HC]).astype(bf),
                "wo": np.ascontiguousarray(Wo[hs : hs + HC, :]).astype(bf),
                "bq": np.ascontiguousarray(bq[hs : hs + HC, None]),
                "bk": np.ascontiguousarray(bk[hs : hs + HC, None]),
                "onesd": np.ones((VW, VW), np.float32),
            }
        )
    return in_maps


def _install_profiling():
    """Reconstruct the NTFF profile hook this container's boot skipped.

    bass_utils' axon trace path wants antenv.axon_hooks (absent here);
    inject a stub module and register the ctypes-based hook from
    trn_agent_boot. Also keep artifacts local (no bucket in container).
    """
    import sys
    import types

    if "antenv.axon_hooks" in sys.modules:
        return
    import antenv  # noqa: F401

    mod = types.ModuleType("antenv.axon_hooks")
    mod._hook = None

    def set_axon_ntff_profile_hook(h):
        mod._hook = h

    def get_axon_ntff_profile_hook():
        return mod._hook

    mod.set_axon_ntff_profile_hook = set_axon_ntff_profile_hook
    mod.get_axon_ntff_profile_hook = get_axon_ntff_profile_hook
    sys.modules["antenv.axon_hooks"] = mod

    from trn_agent_boot.trn_boot import _ntff_profile_via_ctypes

    hook = _ntff_profile_via_ctypes("/opt/axon/libaxon_pjrt.so")
    if hook is not None:
        set_axon_ntff_profile_hook(hook)

    bass_utils.upload_artifacts = lambda tmpdir: tmpdir


def run_device(inputs, trace=False, **kw):
    if trace:
        _install_profiling()
    nc = _get_nc()
    in_maps = make_in_maps(**inputs)
    res = bass_utils.run_bass_kernel_spmd(
        nc, in_maps, list(range(8)), trace=trace, **kw
    )
    return res


def assemble_output(results, Wv_bias_term):
    out = np.zeros((B, SQ, D), np.float32)
    for core in range(8):
        b = core // 2
        out[b] += results[core]["out"]
    out += Wv_bias_term
    return out


def kernel(**inputs):
    res = run_device(inputs)
    bv = np.asarray(inputs["bv"], np.float32)
    bo = np.asarray(inputs["bo"], np.float32)
    Wo = np.asarray(inputs["Wo"], np.float32)
    # attn rows sum to 1, so the bv shift passes straight through attn@v;
    # bv@Wo + bo is added once on the host.
    bias_term = bv @ Wo + bo
    return assemble_output(res.results, bias_term)
